# revision 3
# baseline (speedup 1.0000x reference)
"""MultiHeadAttention (RoPE + causal) on 8 trn2 NeuronCores.

Sharding: data-parallel over batch (2) x tensor-parallel over head-groups
(4 groups of 4 heads). Core c handles batch c//4, heads (c%4)*4..+4.
Each core computes its partial output projection; host sums the 4
partials per batch.

Device layout notes (per core):
  qT/kT/vT    : host-transposed [D=1024, S=2048] bf16
  qwT/kwT     : [d'=256, S] as 2 tiles [128, 2048]  (head-pairs stacked)
  RoPE        : qrot = qw*cos + (R2 @ qw)*sin, R2 = pairwise rotation
  scores      : per head, lhsT=krot[64,k128], rhs=qrot[64,j512] -> [k,j]
  softmax     : exp on ACT (scale=1/8), no max-subtraction (|s|<~8 safe),
                denominator via augmented ones-column in vw (M=65 matmul)
  causal      : k-tiles > j skipped; diagonal tiles masked by 0/1 window
  out proj    : out[j,D] = o^T as lhsT vs Wo tiles; bias via K=1 ones-MM
"""

import numpy as np

B, S, D = 2, 2048, 1024
HEADS, DK = 16, 64
NCORES = 8
GROUPS = 4          # head groups (tensor-parallel)
HPG = HEADS // GROUPS  # 4 heads per group
DG = HPG * DK       # 256 d' per group
NJB = S // 512      # 4 j-blocks of 512
NKT = S // 128      # 16 k-tiles of 128
NDT = D // 128      # 8 D-tiles

_CACHE = {}


def _patch_tile_drain():
    """walrus in this container caps sync-waits at 1 per instruction; the
    stock Tile kernel-tail drain accumulates one wait per logical proc on
    a single Drain. Split them over a chain of SP nops."""
    import bass_rust
    from concourse.tile import TileContext
    from concourse.vector_clock import ScopedClock

    if getattr(TileContext, "_drain_patched", False):
        return

    def _drain_and_barrier(self, tick_clock, wait_clock):
        probe = self.nc.sync.nop(nofuse=True)
        wait_clock.add_sem_waits(
            probe.ins, ScopedClock({None: tick_clock.global_clock})
        )
        si = probe.ins.sync_info
        waits = list(si.on_wait or []) if si else []
        if len(waits) > 1:
            si.on_wait = waits[:1]
            for i in range(1, len(waits)):
                n = self.nc.sync.nop(nofuse=True)
                n.ins.sync_info = bass_rust.SyncInfo(
                    on_wait=waits[i : i + 1], on_update=[]
                )
        self.nc.sync.drain()
        self.nc.all_engine_barrier()
        assert self.sems is not None
        popped = self.nc._tile_sem_poison_stack.pop()
        assert popped is self._sem_poison
        self.nc.clear_and_free_semaphores(list(self.sems.allocated().values()))
        self.nc.all_engine_barrier()

    TileContext._drain_and_barrier = _drain_and_barrier

    # walrus also rejects >1 sync-wait on regular instructions: split the
    # extras onto same-engine InstNoOps placed just before, preserving
    # per-engine wait-then-execute order.
    from concourse import mybir as _mybir

    _orig_lower = TileContext._lower_ordered_insts

    def _lower_ordered_insts(self, ordered):
        Unassigned = _mybir.EngineType.Unassigned
        for bb_name, insts in ordered.items():
            new_list = []
            for inst in insts:
                si = inst.sync_info
                waits = list(si.on_wait or []) if si else []
                if len(waits) > 1 and inst.engine != Unassigned:
                    for w in waits[:-1]:
                        new_list.append(
                            _mybir.InstNoOp(
                                name=self.nc.get_next_instruction_name(),
                                sync_info=_mybir.SyncInfo(
                                    on_wait=[w], on_update=[]
                                ),
                                engine=inst.engine,
                                bass_nofuse=True,
                            )
                        )
                    si.on_wait = waits[-1:]
                new_list.append(inst)
            ordered[bb_name] = new_list
        return _orig_lower(self, ordered)

    TileContext._lower_ordered_insts = _lower_ordered_insts
    TileContext._drain_patched = True


def build_nc():
    import concourse.bass as bass
    import concourse.tile as tile
    from concourse import mybir

    _patch_tile_drain()
    bf = mybir.dt.bfloat16
    f32 = mybir.dt.float32
    AF = mybir.ActivationFunctionType
    ALU = mybir.AluOpType
    PSUM = bass.MemorySpace.PSUM

    nc = bass.Bass()
    dp = lambda n, shp, dt: nc.declare_dram_parameter(n, shp, dt, isOutput=False)
    qT_d = dp("qT", [D, S], bf)
    kT_d = dp("kT", [D, S], bf)
    vT_d = dp("vT", [D, S], bf)
    wq_d = dp("wq", [128, NDT * DG], bf)      # [128, 2048] D-tile-major
    wk_d = dp("wk", [128, NDT * DG], bf)
    wv_d = dp("wv", [128, NDT * 260], bf)     # per-head 65-col aug layout
    wo_d = dp("wo", [128, 2 * D], bf)         # 2 d'-tiles of [128, 1024]
    r2_d = dp("r2", [128, 128], bf)
    cos_d = dp("cosT", [128, S], bf)
    sin_d = dp("sinT", [128, S], bf)
    m01_d = dp("m01", [128, 896], bf)
    bq_d = dp("bq_l", [1, DG], bf)
    bk_d = dp("bk_l", [1, DG], bf)
    bva_d = dp("bv_aug", [1, 260], bf)        # [bv | 1.0 at aug cols]
    bo_d = dp("bo_r", [1, D], bf)             # bo / 4
    vm_d = dp("vm", [128, NKT], f32)          # v_mask, k-tile-major cols
    out_d = nc.declare_dram_parameter("out", [S, D], f32, isOutput=True)

    with tile.TileContext(nc) as tc:
        with (
            tc.tile_pool(name="const", bufs=1) as cp,
            tc.tile_pool(name="data", bufs=1) as dpool,
        ):
            def cload(dram, shp, dt, tag):
                t = cp.tile(shp, dt, tag=tag, name=tag)
                nc.sync.dma_start(out=t[:], in_=dram[:])
                return t

            wq_s = cload(wq_d, [128, NDT * DG], bf, "wq")
            wk_s = cload(wk_d, [128, NDT * DG], bf, "wk")
            wv_s = cload(wv_d, [128, NDT * 260], bf, "wv")
            wo_s = cload(wo_d, [128, 2 * D], bf, "wo")
            r2_s = cload(r2_d, [128, 128], bf, "r2")
            cos_s = cload(cos_d, [128, S], bf, "cos")
            sin_s = cload(sin_d, [128, S], bf, "sin")
            m01_s = cload(m01_d, [128, 896], bf, "m01")
            bq_s = cload(bq_d, [1, DG], bf, "bq")
            bk_s = cload(bk_d, [1, DG], bf, "bk")
            bva_s = cload(bva_d, [1, 260], bf, "bva")
            bo_s = cload(bo_d, [1, D], bf, "bo")
            vm_s = cload(vm_d, [128, NKT], f32, "vm")
            ones_b = cp.tile([1, 512], bf, tag="ones_b")
            nc.vector.memset(ones_b[:], 1.0)
            ones_f = cp.tile([1, 64], f32, tag="ones_f")
            nc.vector.memset(ones_f[:], 1.0)

            # streamed inputs, resident for the projection phase
            qT_s = []
            kT_s = []
            vT_s = []
            for dt_i in range(NDT):
                for lst, dram, nm in (
                    (qT_s, qT_d, "qT"), (kT_s, kT_d, "kT"), (vT_s, vT_d, "vT"),
                ):
                    t = dpool.tile([128, S], bf, tag=f"{nm}{dt_i}", name=f"{nm}{dt_i}")
                    nc.sync.dma_start(
                        out=t[:], in_=dram[dt_i * 128 : (dt_i + 1) * 128, :]
                    )
                    lst.append(t)

            # persistent intermediates
            qrot = [dpool.tile([128, S], bf, tag=f"qrot{i}", name=f"qrot{i}") for i in range(2)]
            krot = [dpool.tile([128, S], bf, tag=f"krot{i}", name=f"krot{i}") for i in range(2)]
            vw = [dpool.tile([128, 260], bf, tag=f"vw{i}", name=f"vw{i}") for i in range(NKT)]
            ot = [dpool.tile([128, S], bf, tag=f"ot{i}", name=f"ot{i}") for i in range(2)]

            # ---------------- Phase A: projections + RoPE ----------------
            with (
                tc.tile_pool(name="psq", bufs=4, space=PSUM) as psq,
                tc.tile_pool(name="psr", bufs=2, space=PSUM) as psr,
                tc.tile_pool(name="psv", bufs=2, space=PSUM) as psv,
                tc.tile_pool(name="tmpA", bufs=3) as tmp,
            ):
                for (xT_s, w_s, b_s, rot) in (
                    (qT_s, wq_s, bq_s, qrot),
                    (kT_s, wk_s, bk_s, krot),
                ):
                    for sb in range(NJB):
                        ssl = slice(sb * 512, (sb + 1) * 512)
                        for blk in range(2):
                            ps = psq.tile([128, 512], f32, tag="proj")
                            nc.tensor.matmul(
                                out=ps[:],
                                lhsT=b_s[0:1, blk * 128 : (blk + 1) * 128],
                                rhs=ones_b[0:1, 0:512],
                                start=True, stop=False,
                            )
                            for dt_i in range(NDT):
                                c0 = dt_i * DG + blk * 128
                                nc.tensor.matmul(
                                    out=ps[:],
                                    lhsT=w_s[:, c0 : c0 + 128],
                                    rhs=xT_s[dt_i][:, ssl],
                                    start=False, stop=(dt_i == NDT - 1),
                                )
                            xw = tmp.tile([128, 512], bf, tag="xw")
                            nc.vector.tensor_copy(out=xw[:], in_=ps[:])
                            pr = psr.tile([128, 512], f32, tag="rot")
                            nc.tensor.matmul(
                                out=pr[:], lhsT=r2_s[:], rhs=xw[:],
                                start=True, stop=True,
                            )
                            xw2 = tmp.tile([128, 512], bf, tag="xw2")
                            nc.vector.tensor_copy(out=xw2[:], in_=pr[:])
                            t1 = tmp.tile([128, 512], bf, tag="t1")
                            nc.vector.tensor_tensor(
                                out=t1[:], in0=xw[:], in1=cos_s[:, ssl],
                                op=ALU.mult,
                            )
                            t2 = tmp.tile([128, 512], bf, tag="t2")
                            nc.vector.tensor_tensor(
                                out=t2[:], in0=xw2[:], in1=sin_s[:, ssl],
                                op=ALU.mult,
                            )
                            nc.vector.tensor_tensor(
                                out=rot[blk][:, ssl], in0=t1[:], in1=t2[:],
                                op=ALU.add,
                            )

                for kt in range(NKT):
                    ksl = slice(kt * 128, (kt + 1) * 128)
                    pv = psv.tile([128, 260], f32, tag="pv")
                    nc.tensor.matmul(
                        out=pv[:], lhsT=ones_b[0:1, 0:128], rhs=bva_s[0:1, :],
                        start=True, stop=False,
                    )
                    for dt_i in range(NDT):
                        nc.tensor.matmul(
                            out=pv[:],
                            lhsT=vT_s[dt_i][:, ksl],
                            rhs=wv_s[:, dt_i * 260 : (dt_i + 1) * 260],
                            start=False, stop=(dt_i == NDT - 1),
                        )
                    nc.vector.tensor_scalar(
                        out=vw[kt][:], in0=pv[:],
                        scalar1=vm_s[:, kt : kt + 1], scalar2=None,
                        op0=ALU.mult,
                    )

            # ---------------- Phase B: attention ----------------
            with (
                tc.tile_pool(name="psc", bufs=2, space=PSUM) as psc,
                tc.tile_pool(name="psa", bufs=2, space=PSUM) as psa,
                tc.tile_pool(name="psb", bufs=1, space=PSUM) as psb,
                tc.tile_pool(name="esb", bufs=3) as esb,
                tc.tile_pool(name="tmpB", bufs=2) as tmpb,
            ):
                for h in range(HPG):
                    hb, hr = h // 2, (h % 2) * 64
                    for jb in range(NJB):
                        jsl = slice(jb * 512, (jb + 1) * 512)
                        nkt_j = 4 * (jb + 1)
                        av = psa.tile([65, 512], f32, tag="av")
                        for kp in range(nkt_j // 2):
                            sc = psc.tile([128, 1024], f32, tag="sc")
                            e = esb.tile([128, 1024], bf, tag="e")
                            for half in range(2):
                                kt = 2 * kp + half
                                nc.tensor.matmul(
                                    out=sc[:, half * 512 : (half + 1) * 512],
                                    lhsT=krot[hb][
                                        hr : hr + 64, kt * 128 : (kt + 1) * 128
                                    ],
                                    rhs=qrot[hb][hr : hr + 64, jsl],
                                    start=True, stop=True,
                                )
                            nc.scalar.activation(
                                out=e[:], in_=sc[:], func=AF.Exp, scale=0.125
                            )
                            for half in range(2):
                                kt = 2 * kp + half
                                off = kt * 128 - jb * 512
                                if off >= 0:  # diagonal tile: 0/1 window mask
                                    st = 384 - off
                                    nc.vector.tensor_tensor(
                                        out=e[:, half * 512 : (half + 1) * 512],
                                        in0=e[:, half * 512 : (half + 1) * 512],
                                        in1=m01_s[:, st : st + 512],
                                        op=ALU.mult,
                                    )
                                nc.tensor.matmul(
                                    out=av[:],
                                    lhsT=vw[kt][:, h * 65 : (h + 1) * 65],
                                    rhs=e[:, half * 512 : (half + 1) * 512],
                                    start=(kt == 0), stop=(kt == nkt_j - 1),
                                )
                        rcp = tmpb.tile([1, 512], f32, tag="rcp")
                        nc.vector.reciprocal(out=rcp[:], in_=av[64:65, :])
                        bc = psb.tile([64, 512], f32, tag="bc")
                        nc.tensor.matmul(
                            out=bc[:], lhsT=ones_f[:], rhs=rcp[:],
                            start=True, stop=True,
                        )
                        bcs = tmpb.tile([64, 512], f32, tag="bcs")
                        nc.vector.tensor_copy(out=bcs[:], in_=bc[:])
                        nc.vector.tensor_tensor(
                            out=ot[hb][hr : hr + 64, jsl],
                            in0=av[0:64, :], in1=bcs[:], op=ALU.mult,
                        )

            # ---------------- Phase C: output projection ----------------
            with (
                tc.tile_pool(name="pso", bufs=3, space=PSUM) as pso,
                tc.tile_pool(name="ost", bufs=3) as ost,
            ):
                for jt in range(S // 128):
                    jsl = slice(jt * 128, (jt + 1) * 128)
                    po = pso.tile([128, D], f32, tag="po")
                    for half in range(2):
                        osl = slice(half * 512, (half + 1) * 512)
                        nc.tensor.matmul(
                            out=po[:, osl],
                            lhsT=ones_b[0:1, 0:128],
                            rhs=bo_s[0:1, osl],
                            start=True, stop=False,
                        )
                        for dt_i in range(2):
                            nc.tensor.matmul(
                                out=po[:, osl],
                                lhsT=ot[dt_i][:, jsl],
                                rhs=wo_s[:, dt_i * D + half * 512 : dt_i * D + (half + 1) * 512],
                                start=False, stop=(dt_i == 1),
                            )
                    ob = ost.tile([128, D], f32, tag="ob")
                    nc.vector.tensor_copy(out=ob[:], in_=po[:])
                    nc.sync.dma_start(out=out_d[jsl, :], in_=ob[:])

    return nc


def _get_nc():
    if "nc" not in _CACHE:
        _CACHE["nc"] = build_nc()
    return _CACHE["nc"]


def _host_prep(inputs):
    import ml_dtypes

    bf = ml_dtypes.bfloat16
    q, k, v = inputs["q"], inputs["k"], inputs["v"]
    rope, v_mask = inputs["rope"], inputs["v_mask"]
    Wq, bq = inputs["Wq"], inputs["bq"]
    Wk, bk = inputs["Wk"], inputs["bk"]
    Wv, bv = inputs["Wv"], inputs["bv"]
    Wo, bo = inputs["Wo"], inputs["bo"]

    def pack_w(w, width):  # [1024, G*width] -> [128, NDT*width]
        return np.ascontiguousarray(
            w.reshape(NDT, 128, width).transpose(1, 0, 2).reshape(128, NDT * width)
        ).astype(bf)

    # pairwise rotation matrix: qw2 = R2^T-applied; R2[2i+1, 2i]=-1, R2[2i,2i+1]=1
    R2 = np.zeros((128, 128), np.float32)
    idx = np.arange(0, 128, 2)
    R2[idx + 1, idx] = -1.0
    R2[idx, idx + 1] = 1.0
    R2 = R2.astype(bf)

    t = np.arange(896, dtype=np.float32)[None, :]
    p = np.arange(128, dtype=np.float32)[:, None]
    M01 = (t >= p + 384.0).astype(np.float32).astype(bf)

    in_maps = []
    for core in range(NCORES):
        b, g = core // GROUPS, core % GROUPS
        gsl = slice(g * DG, (g + 1) * DG)
        cos64 = np.repeat(rope[b, :, 1::2].T, 2, axis=0)  # [64, S]
        sin64 = np.repeat(rope[b, :, 0::2].T, 2, axis=0)
        wv_g = Wv[:, gsl]
        wv_pack = np.zeros((D, 260), np.float32)
        bva = np.zeros((1, 260), np.float32)
        for h in range(HPG):
            wv_pack[:, h * 65 : h * 65 + 64] = wv_g[:, h * 64 : (h + 1) * 64]
            bva[0, h * 65 : h * 65 + 64] = bv[gsl][h * 64 : (h + 1) * 64]
            bva[0, h * 65 + 64] = 1.0
        wo_g = Wo[gsl, :]
        in_maps.append({
            "qT": np.ascontiguousarray(q[b].T).astype(bf),
            "kT": np.ascontiguousarray(k[b].T).astype(bf),
            "vT": np.ascontiguousarray(v[b].T).astype(bf),
            "wq": pack_w(Wq[:, gsl], DG),
            "wk": pack_w(Wk[:, gsl], DG),
            "wv": pack_w(wv_pack, 260),
            "wo": np.ascontiguousarray(
                wo_g.reshape(2, 128, D).transpose(1, 0, 2).reshape(128, 2 * D)
            ).astype(bf),
            "r2": R2,
            "cosT": np.vstack([cos64, cos64]).astype(bf),
            "sinT": np.vstack([sin64, sin64]).astype(bf),
            "m01": M01,
            "bq_l": bq[gsl].reshape(1, DG).astype(bf),
            "bk_l": bk[gsl].reshape(1, DG).astype(bf),
            "bv_aug": bva.astype(bf),
            "bo_r": (bo / GROUPS).reshape(1, D).astype(bf),
            "vm": np.ascontiguousarray(v_mask[b].reshape(NKT, 128).T).astype(
                np.float32
            ),
        })
    return in_maps


def kernel(**inputs):
    from concourse.bass_utils import run_bass_kernel_spmd

    nc = _get_nc()
    in_maps = _host_prep(inputs)
    res = run_bass_kernel_spmd(nc, in_maps, list(range(NCORES)))
    outs = [res.results[i]["out"] for i in range(NCORES)]
    full = np.empty((B, S, D), np.float32)
    for b in range(B):
        full[b] = sum(
            outs[b * GROUPS + g].astype(np.float32) for g in range(GROUPS)
        )
    return full


# revision 9
# speedup vs baseline: 1.0503x; 1.0503x over previous
"""MultiHeadAttention (RoPE + causal) on 8 trn2 NeuronCores.

Sharding: data-parallel over batch (2) x tensor-parallel over head-groups
(4 groups of 4 heads). Core c handles batch c//4, heads (c%4)*4..+4.
Each core computes its partial output projection; host sums the 4
partials per batch.

Device layout notes (per core):
  qT/kT/vT    : host-transposed [D=1024, S=2048] bf16
  qwT/kwT     : [d'=256, S] as 2 tiles [128, 2048]  (head-pairs stacked)
  RoPE        : qrot = qw*cos + (R2 @ qw)*sin, R2 = pairwise rotation
  scores      : per head, lhsT=krot[64,k128], rhs=qrot[64,j512] -> [k,j]
  softmax     : exp on ACT (scale=1/8), no max-subtraction (|s|<~8 safe),
                denominator via augmented ones-column in vw (M=65 matmul)
  causal      : k-tiles > j skipped; diagonal tiles masked by 0/1 window
  out proj    : out[j,D] = o^T as lhsT vs Wo tiles; bias via K=1 ones-MM
"""

import numpy as np

B, S, D = 2, 2048, 1024
HEADS, DK = 16, 64
NCORES = 8
GROUPS = 4          # head groups (tensor-parallel)
HPG = HEADS // GROUPS  # 4 heads per group
DG = HPG * DK       # 256 d' per group
NJB = S // 512      # 4 j-blocks of 512
NKT = S // 128      # 16 k-tiles of 128
NDT = D // 128      # 8 D-tiles

_CACHE = {}


def _patch_tile_drain():
    """walrus in this container caps sync-waits at 1 per instruction; the
    stock Tile kernel-tail drain accumulates one wait per logical proc on
    a single Drain. Split them over a chain of SP nops."""
    import bass_rust
    from concourse.tile import TileContext
    from concourse.vector_clock import ScopedClock

    if getattr(TileContext, "_drain_patched", False):
        return

    def _drain_and_barrier(self, tick_clock, wait_clock):
        probe = self.nc.sync.nop(nofuse=True)
        wait_clock.add_sem_waits(
            probe.ins, ScopedClock({None: tick_clock.global_clock})
        )
        si = probe.ins.sync_info
        waits = list(si.on_wait or []) if si else []
        if len(waits) > 1:
            si.on_wait = waits[:1]
            for i in range(1, len(waits)):
                n = self.nc.sync.nop(nofuse=True)
                n.ins.sync_info = bass_rust.SyncInfo(
                    on_wait=waits[i : i + 1], on_update=[]
                )
        self.nc.sync.drain()
        self.nc.all_engine_barrier()
        assert self.sems is not None
        popped = self.nc._tile_sem_poison_stack.pop()
        assert popped is self._sem_poison
        self.nc.clear_and_free_semaphores(list(self.sems.allocated().values()))
        self.nc.all_engine_barrier()

    TileContext._drain_and_barrier = _drain_and_barrier

    # walrus also rejects >1 sync-wait on regular instructions: split the
    # extras onto same-engine InstNoOps placed just before, preserving
    # per-engine wait-then-execute order.
    from concourse import mybir as _mybir

    _orig_lower = TileContext._lower_ordered_insts

    def _lower_ordered_insts(self, ordered):
        Unassigned = _mybir.EngineType.Unassigned
        for bb_name, insts in ordered.items():
            new_list = []
            for inst in insts:
                si = inst.sync_info
                waits = list(si.on_wait or []) if si else []
                if len(waits) > 1 and inst.engine != Unassigned:
                    for w in waits[:-1]:
                        new_list.append(
                            _mybir.InstNoOp(
                                name=self.nc.get_next_instruction_name(),
                                sync_info=_mybir.SyncInfo(
                                    on_wait=[w], on_update=[]
                                ),
                                engine=inst.engine,
                                bass_nofuse=True,
                            )
                        )
                    si.on_wait = waits[-1:]
                new_list.append(inst)
            ordered[bb_name] = new_list
        return _orig_lower(self, ordered)

    TileContext._lower_ordered_insts = _lower_ordered_insts
    TileContext._drain_patched = True


def build_nc():
    import concourse.bass as bass
    import concourse.tile as tile
    from concourse import mybir

    _patch_tile_drain()
    bf = mybir.dt.bfloat16
    f32 = mybir.dt.float32
    AF = mybir.ActivationFunctionType
    ALU = mybir.AluOpType
    PSUM = bass.MemorySpace.PSUM

    nc = bass.Bass()
    dp = lambda n, shp, dt: nc.declare_dram_parameter(n, shp, dt, isOutput=False)
    qT_d = dp("qT", [D, S], bf)
    kT_d = dp("kT", [D, S], bf)
    vT_d = dp("vT", [D, S], bf)
    wq_d = dp("wq", [128, NDT * DG], bf)      # [128, 2048] D-tile-major
    wk_d = dp("wk", [128, NDT * DG], bf)
    wv_d = dp("wv", [128, NDT * 260], bf)     # per-head 65-col aug layout
    wo_d = dp("wo", [128, 2 * D], bf)         # 2 d'-tiles of [128, 1024]
    r2_d = dp("r2", [128, 128], bf)
    cos_d = dp("cosT", [128, S], bf)
    sin_d = dp("sinT", [128, S], bf)
    m01_d = dp("m01", [128, 896], bf)
    bq_d = dp("bq_l", [1, DG], bf)
    bk_d = dp("bk_l", [1, DG], bf)
    bva_d = dp("bv_aug", [1, 260], bf)        # [bv | 1.0 at aug cols]
    bo_d = dp("bo_r", [1, D], bf)             # bo / 4
    vm_d = dp("vm", [128, NKT], f32)          # v_mask, k-tile-major cols
    out_d = nc.declare_dram_parameter("out", [S, D], f32, isOutput=True)

    with tile.TileContext(nc) as tc:
        with (
            tc.tile_pool(name="const", bufs=1) as cp,
            tc.tile_pool(name="data", bufs=1) as dpool,
        ):
            def cload(dram, shp, dt, tag):
                t = cp.tile(shp, dt, tag=tag, name=tag)
                nc.sync.dma_start(out=t[:], in_=dram[:])
                return t

            wq_s = cload(wq_d, [128, NDT * DG], bf, "wq")
            wk_s = cload(wk_d, [128, NDT * DG], bf, "wk")
            wv_s = cload(wv_d, [128, NDT * 260], bf, "wv")
            wo_s = cload(wo_d, [128, 2 * D], bf, "wo")
            r2_s = cload(r2_d, [128, 128], bf, "r2")
            cos_s = cload(cos_d, [128, S], bf, "cos")
            sin_s = cload(sin_d, [128, S], bf, "sin")
            m01_s = cload(m01_d, [128, 896], bf, "m01")
            bq_s = cload(bq_d, [1, DG], bf, "bq")
            bk_s = cload(bk_d, [1, DG], bf, "bk")
            bva_s = cload(bva_d, [1, 260], bf, "bva")
            bo_s = cload(bo_d, [1, D], bf, "bo")
            vm_s = cload(vm_d, [128, NKT], f32, "vm")
            ones_b = cp.tile([1, 512], bf, tag="ones_b")
            nc.vector.memset(ones_b[:], 1.0)
            ones_f = cp.tile([1, 64], f32, tag="ones_f")
            nc.vector.memset(ones_f[:], 1.0)

            # streamed inputs, resident for the projection phase
            qT_s = []
            kT_s = []
            vT_s = []
            for dt_i in range(NDT):
                for lst, dram, nm in (
                    (qT_s, qT_d, "qT"), (kT_s, kT_d, "kT"), (vT_s, vT_d, "vT"),
                ):
                    t = dpool.tile([128, S], bf, tag=f"{nm}{dt_i}", name=f"{nm}{dt_i}")
                    nc.sync.dma_start(
                        out=t[:], in_=dram[dt_i * 128 : (dt_i + 1) * 128, :]
                    )
                    lst.append(t)

            # persistent intermediates
            qrot = [dpool.tile([128, S], bf, tag=f"qrot{i}", name=f"qrot{i}") for i in range(2)]
            krot = [dpool.tile([128, S], bf, tag=f"krot{i}", name=f"krot{i}") for i in range(2)]
            vw = [dpool.tile([128, 260], bf, tag=f"vw{i}", name=f"vw{i}") for i in range(NKT)]
            ot = [dpool.tile([128, S], bf, tag=f"ot{i}", name=f"ot{i}") for i in range(2)]

            # ---------------- Phase A: projections + RoPE ----------------
            with (
                tc.tile_pool(name="psq", bufs=4, space=PSUM) as psq,
                tc.tile_pool(name="psr", bufs=2, space=PSUM) as psr,
                tc.tile_pool(name="psv", bufs=2, space=PSUM) as psv,
                tc.tile_pool(name="tmpA", bufs=3) as tmp,
            ):
                for (xT_s, w_s, b_s, rot) in (
                    (qT_s, wq_s, bq_s, qrot),
                    (kT_s, wk_s, bk_s, krot),
                ):
                    for blk in range(2):
                        # D-tile-outer: one weight load serves all 4 s-blocks
                        pss = [
                            psq.tile([128, 512], f32, tag="proj", name=f"pj{sb}")
                            for sb in range(NJB)
                        ]
                        for sb in range(NJB):
                            nc.tensor.matmul(
                                out=pss[sb][:],
                                lhsT=b_s[0:1, blk * 128 : (blk + 1) * 128],
                                rhs=ones_b[0:1, 0:512],
                                start=True, stop=False,
                            )
                        for dt_i in range(NDT):
                            c0 = dt_i * DG + blk * 128
                            for sb in range(NJB):
                                nc.tensor.matmul(
                                    out=pss[sb][:],
                                    lhsT=w_s[:, c0 : c0 + 128],
                                    rhs=xT_s[dt_i][:, sb * 512 : (sb + 1) * 512],
                                    start=False, stop=(dt_i == NDT - 1),
                                )
                        for sb in range(NJB):
                            ssl = slice(sb * 512, (sb + 1) * 512)
                            ps = pss[sb]
                            xw = tmp.tile([128, 512], bf, tag="xw")
                            nc.vector.tensor_copy(out=xw[:], in_=ps[:])
                            pr = psr.tile([128, 512], f32, tag="rot")
                            nc.tensor.matmul(
                                out=pr[:], lhsT=r2_s[:], rhs=xw[:],
                                start=True, stop=True,
                            )
                            xw2 = tmp.tile([128, 512], bf, tag="xw2")
                            nc.scalar.activation(
                                out=xw2[:], in_=pr[:], func=AF.Copy
                            )
                            t1 = tmp.tile([128, 512], bf, tag="t1")
                            nc.vector.tensor_tensor(
                                out=t1[:], in0=xw[:], in1=cos_s[:, ssl],
                                op=ALU.mult,
                            )
                            t2 = tmp.tile([128, 512], bf, tag="t2")
                            nc.vector.tensor_tensor(
                                out=t2[:], in0=xw2[:], in1=sin_s[:, ssl],
                                op=ALU.mult,
                            )
                            nc.vector.tensor_tensor(
                                out=rot[blk][:, ssl], in0=t1[:], in1=t2[:],
                                op=ALU.add,
                            )

                for kt in range(NKT):
                    ksl = slice(kt * 128, (kt + 1) * 128)
                    pv = psv.tile([128, 260], f32, tag="pv")
                    nc.tensor.matmul(
                        out=pv[:], lhsT=ones_b[0:1, 0:128], rhs=bva_s[0:1, :],
                        start=True, stop=False,
                    )
                    for dt_i in range(NDT):
                        nc.tensor.matmul(
                            out=pv[:],
                            lhsT=vT_s[dt_i][:, ksl],
                            rhs=wv_s[:, dt_i * 260 : (dt_i + 1) * 260],
                            start=False, stop=(dt_i == NDT - 1),
                        )
                    nc.vector.tensor_scalar(
                        out=vw[kt][:], in0=pv[:],
                        scalar1=vm_s[:, kt : kt + 1], scalar2=None,
                        op0=ALU.mult,
                    )

            # ---------------- Phase B: attention ----------------
            with (
                tc.tile_pool(name="psc", bufs=2, space=PSUM) as psc,
                tc.tile_pool(name="psa", bufs=4, space=PSUM) as psa,
                tc.tile_pool(name="esb", bufs=3) as esb,
                tc.tile_pool(name="tmpB", bufs=3) as tmpb,
            ):
                for h in range(HPG):
                    hb, hr = h // 2, (h % 2) * 64
                    avs = [
                        psa.tile([65, 512], f32, tag="av", name=f"av{jb}")
                        for jb in range(NJB)
                    ]
                    # k-tile-outer: one stationary krot/vw load per k-tile
                    for kt in range(NKT):
                        jb0 = kt // 4  # first causally-valid j-block
                        off = kt * 128 - jb0 * 512  # diag offset in {0,..384}
                        for jp0 in range(jb0, NJB, 2):  # j-block pairs
                            njb_g = min(2, NJB - jp0)
                            w = njb_g * 512
                            sc = psc.tile([128, 1024], f32, tag="sc")
                            for g in range(njb_g):
                                jb = jp0 + g
                                nc.tensor.matmul(
                                    out=sc[:, g * 512 : (g + 1) * 512],
                                    lhsT=krot[hb][
                                        hr : hr + 64, kt * 128 : (kt + 1) * 128
                                    ],
                                    rhs=qrot[hb][
                                        hr : hr + 64, jb * 512 : (jb + 1) * 512
                                    ],
                                    start=True, stop=True,
                                )
                            e = esb.tile([128, 1024], bf, tag="e")
                            nc.scalar.activation(
                                out=e[:, 0:w], in_=sc[:, 0:w],
                                func=AF.Exp, scale=0.125,
                            )
                            if jp0 == jb0:  # diagonal tile: 0/1 window mask
                                nc.vector.tensor_tensor(
                                    out=e[:, 0:512],
                                    in0=e[:, 0:512],
                                    in1=m01_s[:, 384 - off : 896 - off],
                                    op=ALU.mult,
                                )
                            for g in range(njb_g):
                                jb = jp0 + g
                                nc.tensor.matmul(
                                    out=avs[jb][:],
                                    lhsT=vw[kt][:, h * 65 : (h + 1) * 65],
                                    rhs=e[:, g * 512 : (g + 1) * 512],
                                    start=(kt == 0), stop=(kt == 4 * jb + 3),
                                )
                    for jb in range(NJB):
                        jsl = slice(jb * 512, (jb + 1) * 512)
                        rcp = tmpb.tile([1, 512], f32, tag="rcp")
                        nc.vector.reciprocal(out=rcp[:], in_=avs[jb][64:65, :])
                        bc = psc.tile([64, 512], f32, tag="sc", name="bc")
                        nc.tensor.matmul(
                            out=bc[:], lhsT=ones_f[:], rhs=rcp[:],
                            start=True, stop=True,
                        )
                        bcs = tmpb.tile([64, 512], f32, tag="bcs")
                        nc.vector.tensor_copy(out=bcs[:], in_=bc[:])
                        nc.vector.tensor_tensor(
                            out=ot[hb][hr : hr + 64, jsl],
                            in0=avs[jb][0:64, :], in1=bcs[:], op=ALU.mult,
                        )

            # ---------------- Phase C: output projection ----------------
            with (
                tc.tile_pool(name="pso", bufs=3, space=PSUM) as pso,
                tc.tile_pool(name="ost", bufs=3) as ost,
            ):
                for jt in range(S // 128):
                    jsl = slice(jt * 128, (jt + 1) * 128)
                    po = pso.tile([128, D], f32, tag="po")
                    for half in range(2):
                        osl = slice(half * 512, (half + 1) * 512)
                        nc.tensor.matmul(
                            out=po[:, osl],
                            lhsT=ones_b[0:1, 0:128],
                            rhs=bo_s[0:1, osl],
                            start=True, stop=False,
                        )
                        for dt_i in range(2):
                            nc.tensor.matmul(
                                out=po[:, osl],
                                lhsT=ot[dt_i][:, jsl],
                                rhs=wo_s[:, dt_i * D + half * 512 : dt_i * D + (half + 1) * 512],
                                start=False, stop=(dt_i == 1),
                            )
                    ob = ost.tile([128, D], f32, tag="ob")
                    nc.vector.tensor_copy(out=ob[:], in_=po[:])
                    nc.sync.dma_start(out=out_d[jsl, :], in_=ob[:])

    return nc


def _get_nc():
    if "nc" not in _CACHE:
        _CACHE["nc"] = build_nc()
    return _CACHE["nc"]


def _host_prep(inputs):
    import ml_dtypes

    bf = ml_dtypes.bfloat16
    q, k, v = inputs["q"], inputs["k"], inputs["v"]
    rope, v_mask = inputs["rope"], inputs["v_mask"]
    Wq, bq = inputs["Wq"], inputs["bq"]
    Wk, bk = inputs["Wk"], inputs["bk"]
    Wv, bv = inputs["Wv"], inputs["bv"]
    Wo, bo = inputs["Wo"], inputs["bo"]

    def pack_w(w, width):  # [1024, G*width] -> [128, NDT*width]
        return np.ascontiguousarray(
            w.reshape(NDT, 128, width).transpose(1, 0, 2).reshape(128, NDT * width)
        ).astype(bf)

    # pairwise rotation matrix: qw2 = R2^T-applied; R2[2i+1, 2i]=-1, R2[2i,2i+1]=1
    R2 = np.zeros((128, 128), np.float32)
    idx = np.arange(0, 128, 2)
    R2[idx + 1, idx] = -1.0
    R2[idx, idx + 1] = 1.0
    R2 = R2.astype(bf)

    t = np.arange(896, dtype=np.float32)[None, :]
    p = np.arange(128, dtype=np.float32)[:, None]
    M01 = (t >= p + 384.0).astype(np.float32).astype(bf)

    in_maps = []
    for core in range(NCORES):
        b, g = core // GROUPS, core % GROUPS
        gsl = slice(g * DG, (g + 1) * DG)
        cos64 = np.repeat(rope[b, :, 1::2].T, 2, axis=0)  # [64, S]
        sin64 = np.repeat(rope[b, :, 0::2].T, 2, axis=0)
        wv_g = Wv[:, gsl]
        wv_pack = np.zeros((D, 260), np.float32)
        bva = np.zeros((1, 260), np.float32)
        for h in range(HPG):
            wv_pack[:, h * 65 : h * 65 + 64] = wv_g[:, h * 64 : (h + 1) * 64]
            bva[0, h * 65 : h * 65 + 64] = bv[gsl][h * 64 : (h + 1) * 64]
            bva[0, h * 65 + 64] = 1.0
        wo_g = Wo[gsl, :]
        in_maps.append({
            "qT": np.ascontiguousarray(q[b].T).astype(bf),
            "kT": np.ascontiguousarray(k[b].T).astype(bf),
            "vT": np.ascontiguousarray(v[b].T).astype(bf),
            "wq": pack_w(Wq[:, gsl], DG),
            "wk": pack_w(Wk[:, gsl], DG),
            "wv": pack_w(wv_pack, 260),
            "wo": np.ascontiguousarray(
                wo_g.reshape(2, 128, D).transpose(1, 0, 2).reshape(128, 2 * D)
            ).astype(bf),
            "r2": R2,
            "cosT": np.vstack([cos64, cos64]).astype(bf),
            "sinT": np.vstack([sin64, sin64]).astype(bf),
            "m01": M01,
            "bq_l": bq[gsl].reshape(1, DG).astype(bf),
            "bk_l": bk[gsl].reshape(1, DG).astype(bf),
            "bv_aug": bva.astype(bf),
            "bo_r": (bo / GROUPS).reshape(1, D).astype(bf),
            "vm": np.ascontiguousarray(v_mask[b].reshape(NKT, 128).T).astype(
                np.float32
            ),
        })
    return in_maps


def kernel(**inputs):
    from concourse.bass_utils import run_bass_kernel_spmd

    nc = _get_nc()
    in_maps = _host_prep(inputs)
    res = run_bass_kernel_spmd(nc, in_maps, list(range(NCORES)))
    outs = [res.results[i]["out"] for i in range(NCORES)]
    full = np.empty((B, S, D), np.float32)
    for b in range(B):
        full[b] = sum(
            outs[b * GROUPS + g].astype(np.float32) for g in range(GROUPS)
        )
    return full


# revision 11
# speedup vs baseline: 1.0653x; 1.0143x over previous
"""MultiHeadAttention (RoPE + causal) on 8 trn2 NeuronCores.

Sharding: data-parallel over batch (2) x tensor-parallel over head-groups
(4 groups of 4 heads). Core c handles batch c//4, heads (c%4)*4..+4.
Each core computes its partial output projection; host sums the 4
partials per batch.

Device layout notes (per core):
  qT/kT/vT    : host-transposed [D=1024, S=2048] bf16
  qwT/kwT     : [d'=256, S] as 2 tiles [128, 2048]  (head-pairs stacked)
  RoPE        : qrot = qw*cos + (R2 @ qw)*sin, R2 = pairwise rotation
  scores      : per head, lhsT=krot[64,k128], rhs=qrot[64,j512] -> [k,j]
  softmax     : exp on ACT (scale=1/8), no max-subtraction (|s|<~8 safe),
                denominator via augmented ones-column in vw (M=65 matmul)
  causal      : k-tiles > j skipped; diagonal tiles masked by 0/1 window
  out proj    : out[j,D] = o^T as lhsT vs Wo tiles; bias via K=1 ones-MM
"""

import numpy as np

B, S, D = 2, 2048, 1024
HEADS, DK = 16, 64
NCORES = 8
GROUPS = 4          # head groups (tensor-parallel)
HPG = HEADS // GROUPS  # 4 heads per group
DG = HPG * DK       # 256 d' per group
NJB = S // 512      # 4 j-blocks of 512
NKT = S // 128      # 16 k-tiles of 128
NDT = D // 128      # 8 D-tiles

_CACHE = {}


def _patch_tile_drain():
    """walrus in this container caps sync-waits at 1 per instruction; the
    stock Tile kernel-tail drain accumulates one wait per logical proc on
    a single Drain. Split them over a chain of SP nops."""
    import bass_rust
    from concourse.tile import TileContext
    from concourse.vector_clock import ScopedClock

    if getattr(TileContext, "_drain_patched", False):
        return

    def _drain_and_barrier(self, tick_clock, wait_clock):
        probe = self.nc.sync.nop(nofuse=True)
        wait_clock.add_sem_waits(
            probe.ins, ScopedClock({None: tick_clock.global_clock})
        )
        si = probe.ins.sync_info
        waits = list(si.on_wait or []) if si else []
        if len(waits) > 1:
            si.on_wait = waits[:1]
            for i in range(1, len(waits)):
                n = self.nc.sync.nop(nofuse=True)
                n.ins.sync_info = bass_rust.SyncInfo(
                    on_wait=waits[i : i + 1], on_update=[]
                )
        self.nc.sync.drain()
        self.nc.all_engine_barrier()
        assert self.sems is not None
        popped = self.nc._tile_sem_poison_stack.pop()
        assert popped is self._sem_poison
        self.nc.clear_and_free_semaphores(list(self.sems.allocated().values()))
        self.nc.all_engine_barrier()

    TileContext._drain_and_barrier = _drain_and_barrier

    # walrus also rejects >1 sync-wait on regular instructions: split the
    # extras onto same-engine InstNoOps placed just before, preserving
    # per-engine wait-then-execute order.
    from concourse import mybir as _mybir

    _orig_lower = TileContext._lower_ordered_insts

    def _lower_ordered_insts(self, ordered):
        Unassigned = _mybir.EngineType.Unassigned
        for bb_name, insts in ordered.items():
            new_list = []
            for inst in insts:
                si = inst.sync_info
                waits = list(si.on_wait or []) if si else []
                if len(waits) > 1 and inst.engine != Unassigned:
                    for w in waits[:-1]:
                        new_list.append(
                            _mybir.InstNoOp(
                                name=self.nc.get_next_instruction_name(),
                                sync_info=_mybir.SyncInfo(
                                    on_wait=[w], on_update=[]
                                ),
                                engine=inst.engine,
                                bass_nofuse=True,
                            )
                        )
                    si.on_wait = waits[-1:]
                new_list.append(inst)
            ordered[bb_name] = new_list
        return _orig_lower(self, ordered)

    TileContext._lower_ordered_insts = _lower_ordered_insts
    TileContext._drain_patched = True


def build_nc():
    import concourse.bass as bass
    import concourse.tile as tile
    from concourse import mybir

    _patch_tile_drain()
    bf = mybir.dt.bfloat16
    f32 = mybir.dt.float32
    AF = mybir.ActivationFunctionType
    ALU = mybir.AluOpType
    PSUM = bass.MemorySpace.PSUM

    nc = bass.Bass()
    dp = lambda n, shp, dt: nc.declare_dram_parameter(n, shp, dt, isOutput=False)
    qT_d = dp("qT", [D, S], bf)
    kT_d = dp("kT", [D, S], bf)
    vT_d = dp("vT", [D, S], bf)
    wq_d = dp("wq", [128, NDT * DG], bf)      # [128, 2048] D-tile-major
    wk_d = dp("wk", [128, NDT * DG], bf)
    wv_d = dp("wv", [128, NDT * 260], bf)     # per-head 65-col aug layout
    wo_d = dp("wo", [128, 2 * D], bf)         # 2 d'-tiles of [128, 1024]
    r2_d = dp("r2", [128, 128], bf)
    cos_d = dp("cosT", [128, S], bf)
    sin_d = dp("sinT", [128, S], bf)
    m01_d = dp("m01", [128, 896], bf)
    bq_d = dp("bq_l", [1, DG], bf)
    bk_d = dp("bk_l", [1, DG], bf)
    bva_d = dp("bv_aug", [1, 260], bf)        # [bv | 1.0 at aug cols]
    bo_d = dp("bo_r", [1, D], bf)             # bo / 4
    vm_d = dp("vm", [128, NKT], f32)          # v_mask, k-tile-major cols
    out_d = nc.declare_dram_parameter("out", [S, D], f32, isOutput=True)

    with tile.TileContext(nc) as tc:
        with (
            tc.tile_pool(name="const", bufs=1) as cp,
            tc.tile_pool(name="data", bufs=1) as dpool,
        ):
            def cload(dram, shp, dt, tag):
                t = cp.tile(shp, dt, tag=tag, name=tag)
                nc.sync.dma_start(out=t[:], in_=dram[:])
                return t

            wq_s = cload(wq_d, [128, NDT * DG], bf, "wq")
            wk_s = cload(wk_d, [128, NDT * DG], bf, "wk")
            wv_s = cload(wv_d, [128, NDT * 260], bf, "wv")
            wo_s = cload(wo_d, [128, 2 * D], bf, "wo")
            r2_s = cload(r2_d, [128, 128], bf, "r2")
            cos_s = cload(cos_d, [128, S], bf, "cos")
            sin_s = cload(sin_d, [128, S], bf, "sin")
            m01_s = cload(m01_d, [128, 896], bf, "m01")
            bq_s = cload(bq_d, [1, DG], bf, "bq")
            bk_s = cload(bk_d, [1, DG], bf, "bk")
            bva_s = cload(bva_d, [1, 260], bf, "bva")
            bo_s = cload(bo_d, [1, D], bf, "bo")
            vm_s = cload(vm_d, [128, NKT], f32, "vm")
            ones_b = cp.tile([1, 512], bf, tag="ones_b")
            nc.vector.memset(ones_b[:], 1.0)
            ones_f = cp.tile([1, 64], f32, tag="ones_f")
            nc.vector.memset(ones_f[:], 1.0)

            # streamed inputs, resident for the projection phase
            qT_s = []
            kT_s = []
            vT_s = []
            for dt_i in range(NDT):
                for lst, dram, nm in (
                    (qT_s, qT_d, "qT"), (kT_s, kT_d, "kT"), (vT_s, vT_d, "vT"),
                ):
                    t = dpool.tile([128, S], bf, tag=f"{nm}{dt_i}", name=f"{nm}{dt_i}")
                    nc.sync.dma_start(
                        out=t[:], in_=dram[dt_i * 128 : (dt_i + 1) * 128, :]
                    )
                    lst.append(t)

            # persistent intermediates
            qrot = [dpool.tile([128, S], bf, tag=f"qrot{i}", name=f"qrot{i}") for i in range(2)]
            krot = [dpool.tile([128, S], bf, tag=f"krot{i}", name=f"krot{i}") for i in range(2)]
            vw = [dpool.tile([128, 260], bf, tag=f"vw{i}", name=f"vw{i}") for i in range(NKT)]
            ot = [dpool.tile([128, S], bf, tag=f"ot{i}", name=f"ot{i}") for i in range(2)]

            # Single PSUM pool, tags: proj(2) + sc(2x2) + av(2) = 8 banks
            with (
                tc.tile_pool(name="ps", bufs=2, space=PSUM) as ps,
                tc.tile_pool(name="tmpA", bufs=3) as tmp,
                tc.tile_pool(name="esb", bufs=3) as esb,
                tc.tile_pool(name="tmpB", bufs=3) as tmpb,
                tc.tile_pool(name="ost", bufs=3) as ost,
            ):
                def proj_rope(xT_s, w_s, b_s, rot, blk):
                    for sb in range(NJB):
                        ssl = slice(sb * 512, (sb + 1) * 512)
                        pj = ps.tile([128, 512], f32, tag="proj", name="pj")
                        nc.tensor.matmul(
                            out=pj[:],
                            lhsT=b_s[0:1, blk * 128 : (blk + 1) * 128],
                            rhs=ones_b[0:1, 0:512],
                            start=True, stop=False,
                        )
                        for dt_i in range(NDT):
                            c0 = dt_i * DG + blk * 128
                            nc.tensor.matmul(
                                out=pj[:],
                                lhsT=w_s[:, c0 : c0 + 128],
                                rhs=xT_s[dt_i][:, ssl],
                                start=False, stop=(dt_i == NDT - 1),
                            )
                        xw = tmp.tile([128, 512], bf, tag="xw")
                        nc.vector.tensor_copy(out=xw[:], in_=pj[:])
                        pr = ps.tile([128, 512], f32, tag="proj", name="pr")
                        nc.tensor.matmul(
                            out=pr[:], lhsT=r2_s[:], rhs=xw[:],
                            start=True, stop=True,
                        )
                        xw2 = tmp.tile([128, 512], bf, tag="xw2")
                        nc.scalar.activation(out=xw2[:], in_=pr[:], func=AF.Copy)
                        t1 = tmp.tile([128, 512], bf, tag="t1")
                        nc.vector.tensor_tensor(
                            out=t1[:], in0=xw[:], in1=cos_s[:, ssl], op=ALU.mult
                        )
                        t2 = tmp.tile([128, 512], bf, tag="t2")
                        nc.vector.tensor_tensor(
                            out=t2[:], in0=xw2[:], in1=sin_s[:, ssl], op=ALU.mult
                        )
                        nc.vector.tensor_tensor(
                            out=rot[blk][:, ssl], in0=t1[:], in1=t2[:], op=ALU.add
                        )

                def vw_proj():
                    for kt in range(NKT):
                        ksl = slice(kt * 128, (kt + 1) * 128)
                        pv = ps.tile([128, 260], f32, tag="proj", name="pv")
                        nc.tensor.matmul(
                            out=pv[:], lhsT=ones_b[0:1, 0:128], rhs=bva_s[0:1, :],
                            start=True, stop=False,
                        )
                        for dt_i in range(NDT):
                            nc.tensor.matmul(
                                out=pv[:],
                                lhsT=vT_s[dt_i][:, ksl],
                                rhs=wv_s[:, dt_i * 260 : (dt_i + 1) * 260],
                                start=False, stop=(dt_i == NDT - 1),
                            )
                        nc.vector.tensor_scalar(
                            out=vw[kt][:], in0=pv[:],
                            scalar1=vm_s[:, kt : kt + 1], scalar2=None,
                            op0=ALU.mult,
                        )

                def attention(h):
                    hb, hr = h // 2, (h % 2) * 64
                    for jb in range(NJB):
                        J0 = jb * 512
                        nkt_j = 4 * (jb + 1)
                        av = ps.tile([65, 512], f32, tag="av", name="av")
                        for kp in range(nkt_j // 2):
                            kts = (2 * kp, 2 * kp + 1)
                            # exact-causal: within the diagonal band start
                            # scores at column kt*128 instead of J0
                            offs = [max(0, kt * 128 - J0) for kt in kts]
                            ws = [512 - o for o in offs]
                            col0 = [0, 512] if offs[0] == 0 else [0, ws[0]]
                            sc = ps.tile([128, 1024], f32, tag="sc", name="sc")
                            for i, kt in enumerate(kts):
                                nc.tensor.matmul(
                                    out=sc[:, col0[i] : col0[i] + ws[i]],
                                    lhsT=krot[hb][
                                        hr : hr + 64, kt * 128 : (kt + 1) * 128
                                    ],
                                    rhs=qrot[hb][
                                        hr : hr + 64, J0 + offs[i] : J0 + 512
                                    ],
                                    start=True, stop=True,
                                )
                            e = esb.tile([128, 1024], bf, tag="e")
                            wtot = col0[1] + ws[1]
                            nc.scalar.activation(
                                out=e[:, 0:wtot], in_=sc[:, 0:wtot],
                                func=AF.Exp, scale=0.125,
                            )
                            for i, kt in enumerate(kts):
                                if offs[i] or kt * 128 == J0:  # diagonal tile
                                    nc.gpsimd.tensor_tensor(
                                        out=e[:, col0[i] : col0[i] + ws[i]],
                                        in0=e[:, col0[i] : col0[i] + ws[i]],
                                        in1=m01_s[:, 384 : 384 + ws[i]],
                                        op=ALU.mult,
                                    )
                                nc.tensor.matmul(
                                    out=av[:, offs[i] : 512],
                                    lhsT=vw[kt][:, h * 65 : (h + 1) * 65],
                                    rhs=e[:, col0[i] : col0[i] + ws[i]],
                                    start=(kt == 0), stop=(kt == nkt_j - 1),
                                )
                        rcp = tmpb.tile([1, 512], f32, tag="rcp")
                        nc.vector.reciprocal(out=rcp[:], in_=av[64:65, :])
                        bc = ps.tile([64, 512], f32, tag="av", name="bc")
                        nc.tensor.matmul(
                            out=bc[:], lhsT=ones_f[:], rhs=rcp[:],
                            start=True, stop=True,
                        )
                        bcs = tmpb.tile([64, 512], f32, tag="bcs")
                        nc.scalar.activation(out=bcs[:], in_=bc[:], func=AF.Copy)
                        nc.vector.tensor_tensor(
                            out=ot[hb][hr : hr + 64, J0 : J0 + 512],
                            in0=av[0:64, :], in1=bcs[:], op=ALU.mult,
                        )

                def oproj():
                    for jt in range(S // 128):
                        jsl = slice(jt * 128, (jt + 1) * 128)
                        po = ps.tile([128, D], f32, tag="sc", name="po")
                        for half in range(2):
                            osl = slice(half * 512, (half + 1) * 512)
                            nc.tensor.matmul(
                                out=po[:, osl],
                                lhsT=ones_b[0:1, 0:128],
                                rhs=bo_s[0:1, osl],
                                start=True, stop=False,
                            )
                            for dt_i in range(2):
                                nc.tensor.matmul(
                                    out=po[:, osl],
                                    lhsT=ot[dt_i][:, jsl],
                                    rhs=wo_s[:, dt_i * D + half * 512 : dt_i * D + (half + 1) * 512],
                                    start=False, stop=(dt_i == 1),
                                )
                        ob = ost.tile([128, D], f32, tag="ob")
                        nc.vector.tensor_copy(out=ob[:], in_=po[:])
                        nc.sync.dma_start(out=out_d[jsl, :], in_=ob[:])

                # order: blk0 projections + vw, then attention h0 first so
                # blk1 projection MMs act as PE filler during h0/h1 exp stalls
                proj_rope(qT_s, wq_s, bq_s, qrot, 0)
                proj_rope(kT_s, wk_s, bk_s, krot, 0)
                vw_proj()
                attention(0)
                proj_rope(qT_s, wq_s, bq_s, qrot, 1)
                proj_rope(kT_s, wk_s, bk_s, krot, 1)
                attention(1)
                attention(2)
                attention(3)
                oproj()

    return nc


def _get_nc():
    if "nc" not in _CACHE:
        _CACHE["nc"] = build_nc()
    return _CACHE["nc"]


def _host_prep(inputs):
    import ml_dtypes

    bf = ml_dtypes.bfloat16
    q, k, v = inputs["q"], inputs["k"], inputs["v"]
    rope, v_mask = inputs["rope"], inputs["v_mask"]
    Wq, bq = inputs["Wq"], inputs["bq"]
    Wk, bk = inputs["Wk"], inputs["bk"]
    Wv, bv = inputs["Wv"], inputs["bv"]
    Wo, bo = inputs["Wo"], inputs["bo"]

    def pack_w(w, width):  # [1024, G*width] -> [128, NDT*width]
        return np.ascontiguousarray(
            w.reshape(NDT, 128, width).transpose(1, 0, 2).reshape(128, NDT * width)
        ).astype(bf)

    # pairwise rotation matrix: qw2 = R2^T-applied; R2[2i+1, 2i]=-1, R2[2i,2i+1]=1
    R2 = np.zeros((128, 128), np.float32)
    idx = np.arange(0, 128, 2)
    R2[idx + 1, idx] = -1.0
    R2[idx, idx + 1] = 1.0
    R2 = R2.astype(bf)

    t = np.arange(896, dtype=np.float32)[None, :]
    p = np.arange(128, dtype=np.float32)[:, None]
    M01 = (t >= p + 384.0).astype(np.float32).astype(bf)

    in_maps = []
    for core in range(NCORES):
        b, g = core // GROUPS, core % GROUPS
        gsl = slice(g * DG, (g + 1) * DG)
        cos64 = np.repeat(rope[b, :, 1::2].T, 2, axis=0)  # [64, S]
        sin64 = np.repeat(rope[b, :, 0::2].T, 2, axis=0)
        wv_g = Wv[:, gsl]
        wv_pack = np.zeros((D, 260), np.float32)
        bva = np.zeros((1, 260), np.float32)
        for h in range(HPG):
            wv_pack[:, h * 65 : h * 65 + 64] = wv_g[:, h * 64 : (h + 1) * 64]
            bva[0, h * 65 : h * 65 + 64] = bv[gsl][h * 64 : (h + 1) * 64]
            bva[0, h * 65 + 64] = 1.0
        wo_g = Wo[gsl, :]
        in_maps.append({
            "qT": np.ascontiguousarray(q[b].T).astype(bf),
            "kT": np.ascontiguousarray(k[b].T).astype(bf),
            "vT": np.ascontiguousarray(v[b].T).astype(bf),
            "wq": pack_w(Wq[:, gsl], DG),
            "wk": pack_w(Wk[:, gsl], DG),
            "wv": pack_w(wv_pack, 260),
            "wo": np.ascontiguousarray(
                wo_g.reshape(2, 128, D).transpose(1, 0, 2).reshape(128, 2 * D)
            ).astype(bf),
            "r2": R2,
            "cosT": np.vstack([cos64, cos64]).astype(bf),
            "sinT": np.vstack([sin64, sin64]).astype(bf),
            "m01": M01,
            "bq_l": bq[gsl].reshape(1, DG).astype(bf),
            "bk_l": bk[gsl].reshape(1, DG).astype(bf),
            "bv_aug": bva.astype(bf),
            "bo_r": (bo / GROUPS).reshape(1, D).astype(bf),
            "vm": np.ascontiguousarray(v_mask[b].reshape(NKT, 128).T).astype(
                np.float32
            ),
        })
    return in_maps


def kernel(**inputs):
    from concourse.bass_utils import run_bass_kernel_spmd

    nc = _get_nc()
    in_maps = _host_prep(inputs)
    res = run_bass_kernel_spmd(nc, in_maps, list(range(NCORES)))
    outs = [res.results[i]["out"] for i in range(NCORES)]
    full = np.empty((B, S, D), np.float32)
    for b in range(B):
        full[b] = sum(
            outs[b * GROUPS + g].astype(np.float32) for g in range(GROUPS)
        )
    return full


# revision 12
# speedup vs baseline: 1.3259x; 1.2446x over previous
"""MultiHeadAttention (RoPE + causal) on 8 trn2 NeuronCores.

Sharding: data-parallel over batch (2) x tensor-parallel over head-groups
(4 groups of 4 heads). Core c handles batch c//4, heads (c%4)*4..+4.
Each core computes its partial output projection; host sums the 4
partials per batch.

Device layout notes (per core):
  qT/kT/vT    : host-transposed [D=1024, S=2048] bf16
  qwT/kwT     : [d'=256, S] as 2 tiles [128, 2048]  (head-pairs stacked)
  RoPE        : qrot = qw*cos + (R2 @ qw)*sin, R2 = pairwise rotation
  scores      : per head, lhsT=krot[64,k128], rhs=qrot[64,j512] -> [k,j]
  softmax     : exp on ACT (scale=1/8), no max-subtraction (|s|<~8 safe),
                denominator via augmented ones-column in vw (M=65 matmul)
  causal      : k-tiles > j skipped; diagonal tiles masked by 0/1 window
  out proj    : out[j,D] = o^T as lhsT vs Wo tiles; bias via K=1 ones-MM
"""

import numpy as np

B, S, D = 2, 2048, 1024
HEADS, DK = 16, 64
NCORES = 8
GROUPS = 4          # head groups (tensor-parallel)
HPG = HEADS // GROUPS  # 4 heads per group
DG = HPG * DK       # 256 d' per group
NJB = S // 512      # 4 j-blocks of 512
NKT = S // 128      # 16 k-tiles of 128
NDT = D // 128      # 8 D-tiles

_CACHE = {}


def _patch_tile_drain():
    """walrus in this container caps sync-waits at 1 per instruction; the
    stock Tile kernel-tail drain accumulates one wait per logical proc on
    a single Drain. Split them over a chain of SP nops."""
    import bass_rust
    from concourse.tile import TileContext
    from concourse.vector_clock import ScopedClock

    if getattr(TileContext, "_drain_patched", False):
        return

    def _drain_and_barrier(self, tick_clock, wait_clock):
        probe = self.nc.sync.nop(nofuse=True)
        wait_clock.add_sem_waits(
            probe.ins, ScopedClock({None: tick_clock.global_clock})
        )
        si = probe.ins.sync_info
        waits = list(si.on_wait or []) if si else []
        if len(waits) > 1:
            si.on_wait = waits[:1]
            for i in range(1, len(waits)):
                n = self.nc.sync.nop(nofuse=True)
                n.ins.sync_info = bass_rust.SyncInfo(
                    on_wait=waits[i : i + 1], on_update=[]
                )
        self.nc.sync.drain()
        self.nc.all_engine_barrier()
        assert self.sems is not None
        popped = self.nc._tile_sem_poison_stack.pop()
        assert popped is self._sem_poison
        self.nc.clear_and_free_semaphores(list(self.sems.allocated().values()))
        self.nc.all_engine_barrier()

    TileContext._drain_and_barrier = _drain_and_barrier

    # walrus also rejects >1 sync-wait on regular instructions: split the
    # extras onto same-engine InstNoOps placed just before, preserving
    # per-engine wait-then-execute order.
    from concourse import mybir as _mybir

    _orig_lower = TileContext._lower_ordered_insts

    def _lower_ordered_insts(self, ordered):
        Unassigned = _mybir.EngineType.Unassigned
        for bb_name, insts in ordered.items():
            new_list = []
            for inst in insts:
                si = inst.sync_info
                waits = list(si.on_wait or []) if si else []
                if len(waits) > 1 and inst.engine != Unassigned:
                    for w in waits[:-1]:
                        new_list.append(
                            _mybir.InstNoOp(
                                name=self.nc.get_next_instruction_name(),
                                sync_info=_mybir.SyncInfo(
                                    on_wait=[w], on_update=[]
                                ),
                                engine=inst.engine,
                                bass_nofuse=True,
                            )
                        )
                    si.on_wait = waits[-1:]
                new_list.append(inst)
            ordered[bb_name] = new_list
        return _orig_lower(self, ordered)

    TileContext._lower_ordered_insts = _lower_ordered_insts
    TileContext._drain_patched = True


def build_nc(zero_bias=True):
    import concourse.bass as bass
    import concourse.tile as tile
    from concourse import mybir

    _patch_tile_drain()
    bf = mybir.dt.bfloat16
    f32 = mybir.dt.float32
    AF = mybir.ActivationFunctionType
    ALU = mybir.AluOpType
    PSUM = bass.MemorySpace.PSUM

    nc = bass.Bass()
    dp = lambda n, shp, dt: nc.declare_dram_parameter(n, shp, dt, isOutput=False)
    qT_d = dp("qT", [D, S], bf)
    kT_d = dp("kT", [D, S], bf)
    vT_d = dp("vT", [D, S], bf)
    wq_d = dp("wq", [128, NDT * DG], bf)      # [128, 2048] D-tile-major
    wk_d = dp("wk", [128, NDT * DG], bf)
    wv_d = dp("wv", [128, NDT * 260], bf)     # per-head 65-col aug layout
    wo_d = dp("wo", [128, 2 * D], bf)         # 2 d'-tiles of [128, 1024]
    r2_d = dp("r2", [128, 128], bf)
    cos_d = dp("cosT", [128, S], bf)
    sin_d = dp("sinT", [128, S], bf)
    m01_d = dp("m01", [128, 896], bf)
    bq_d = dp("bq_l", [1, DG], bf)
    bk_d = dp("bk_l", [1, DG], bf)
    bva_d = dp("bv_aug", [1, 260], bf)        # [bv | 1.0 at aug cols]
    bo_d = dp("bo_r", [1, D], bf)             # bo / 4
    vm_d = dp("vm", [128, NKT], f32)          # v_mask, k-tile-major cols
    out_d = nc.declare_dram_parameter("out", [S, D], f32, isOutput=True)

    with tile.TileContext(nc) as tc:
        with (
            tc.tile_pool(name="const", bufs=1) as cp,
            tc.tile_pool(name="data", bufs=1) as dpool,
        ):
            def cload(dram, shp, dt, tag):
                t = cp.tile(shp, dt, tag=tag, name=tag)
                nc.sync.dma_start(out=t[:], in_=dram[:])
                return t

            def stream_in(dram, nm):
                # [D, S] input as [128, 512] chunks, s-major so the first
                # projection psum completes as early as possible
                tiles = [
                    dpool.tile([128, S], bf, tag=f"{nm}{i}", name=f"{nm}{i}")
                    for i in range(NDT)
                ]
                for sb in range(NJB):
                    for dt_i in range(NDT):
                        nc.sync.dma_start(
                            out=tiles[dt_i][:, sb * 512 : (sb + 1) * 512],
                            in_=dram[
                                dt_i * 128 : (dt_i + 1) * 128,
                                sb * 512 : (sb + 1) * 512,
                            ],
                        )
                return tiles

            # loads in need-order; DMA queues drain roughly in issue order
            wq_s = cload(wq_d, [128, NDT * DG], bf, "wq")
            r2_s = cload(r2_d, [128, 128], bf, "r2")
            cos_s = cload(cos_d, [128, S], bf, "cos")
            sin_s = cload(sin_d, [128, S], bf, "sin")
            bq_s = cload(bq_d, [1, DG], bf, "bq")
            ones_b = cp.tile([1, 512], bf, tag="ones_b")
            nc.vector.memset(ones_b[:], 1.0)
            ones_bb = cp.tile([1, 64], bf, tag="ones_bb")
            nc.vector.memset(ones_bb[:], 1.0)
            qT_s = stream_in(qT_d, "qT")
            wk_s = cload(wk_d, [128, NDT * DG], bf, "wk")
            bk_s = cload(bk_d, [1, DG], bf, "bk")
            kT_s = stream_in(kT_d, "kT")
            wv_s = cload(wv_d, [128, NDT * 260], bf, "wv")
            bva_s = cload(bva_d, [1, 260], bf, "bva")
            vm_s = cload(vm_d, [128, NKT], f32, "vm")
            vT_s = stream_in(vT_d, "vT")
            m01_s = cload(m01_d, [128, 896], bf, "m01")
            wo_s = cload(wo_d, [128, 2 * D], bf, "wo")
            bo_s = cload(bo_d, [1, D], bf, "bo")

            # persistent intermediates
            qrot = [dpool.tile([128, S], bf, tag=f"qrot{i}", name=f"qrot{i}") for i in range(2)]
            krot = [dpool.tile([128, S], bf, tag=f"krot{i}", name=f"krot{i}") for i in range(2)]
            vw = [dpool.tile([128, 260], bf, tag=f"vw{i}", name=f"vw{i}") for i in range(NKT)]
            ot = [dpool.tile([128, S], bf, tag=f"ot{i}", name=f"ot{i}") for i in range(2)]

            # Single PSUM pool, tags: proj(2) + sc(2x2) + av(2) = 8 banks
            with (
                tc.tile_pool(name="ps", bufs=2, space=PSUM) as ps,
                tc.tile_pool(name="tmpA", bufs=3) as tmp,
                tc.tile_pool(name="esb", bufs=4) as esb,
                tc.tile_pool(name="tmpB", bufs=3) as tmpb,
                tc.tile_pool(name="ost", bufs=3) as ost,
            ):
                def act_recip(out_ap, in_ap):
                    eng = nc.scalar
                    ins_ = [eng.lower_ap(in_ap)]
                    for arg in (0.0, 1.0, 0.0):
                        ins_.append(
                            mybir.ImmediateValue(dtype=f32, value=arg)
                        )
                    eng.add_instruction(
                        mybir.InstActivation(
                            name=nc.get_next_instruction_name(),
                            func=AF.Reciprocal,
                            ins=ins_,
                            outs=[eng.lower_ap(out_ap)],
                        )
                    )

                def proj_rope(xT_s, w_s, b_s, rot, blk):
                    for sb in range(NJB):
                        ssl = slice(sb * 512, (sb + 1) * 512)
                        pj = ps.tile([128, 512], f32, tag="proj", name="pj")
                        if not zero_bias:
                            nc.tensor.matmul(
                                out=pj[:],
                                lhsT=b_s[0:1, blk * 128 : (blk + 1) * 128],
                                rhs=ones_b[0:1, 0:512],
                                start=True, stop=False,
                            )
                        for dt_i in range(NDT):
                            c0 = dt_i * DG + blk * 128
                            nc.tensor.matmul(
                                out=pj[:],
                                lhsT=w_s[:, c0 : c0 + 128],
                                rhs=xT_s[dt_i][:, ssl],
                                start=(zero_bias and dt_i == 0),
                                stop=(dt_i == NDT - 1),
                            )
                        xw = tmp.tile([128, 512], bf, tag="xw")
                        nc.vector.tensor_copy(out=xw[:], in_=pj[:])
                        pr = ps.tile([128, 512], f32, tag="proj", name="pr")
                        nc.tensor.matmul(
                            out=pr[:], lhsT=r2_s[:], rhs=xw[:],
                            start=True, stop=True,
                        )
                        xw2 = tmp.tile([128, 512], bf, tag="xw2")
                        nc.scalar.activation(out=xw2[:], in_=pr[:], func=AF.Copy)
                        t1 = tmp.tile([128, 512], bf, tag="t1")
                        nc.vector.tensor_tensor(
                            out=t1[:], in0=xw[:], in1=cos_s[:, ssl], op=ALU.mult
                        )
                        t2 = tmp.tile([128, 512], bf, tag="t2")
                        nc.vector.tensor_tensor(
                            out=t2[:], in0=xw2[:], in1=sin_s[:, ssl], op=ALU.mult
                        )
                        nc.vector.tensor_tensor(
                            out=rot[blk][:, ssl], in0=t1[:], in1=t2[:], op=ALU.add
                        )

                def vw_proj():
                    for kt in range(NKT):
                        ksl = slice(kt * 128, (kt + 1) * 128)
                        pv = ps.tile([128, 260], f32, tag="proj", name="pv")
                        nc.tensor.matmul(
                            out=pv[:], lhsT=ones_b[0:1, 0:128], rhs=bva_s[0:1, :],
                            start=True, stop=False,
                        )
                        for dt_i in range(NDT):
                            nc.tensor.matmul(
                                out=pv[:],
                                lhsT=vT_s[dt_i][:, ksl],
                                rhs=wv_s[:, dt_i * 260 : (dt_i + 1) * 260],
                                start=False, stop=(dt_i == NDT - 1),
                            )
                        nc.vector.tensor_scalar(
                            out=vw[kt][:], in0=pv[:],
                            scalar1=vm_s[:, kt : kt + 1], scalar2=None,
                            op0=ALU.mult,
                        )

                def attention(h):
                    hb, hr = h // 2, (h % 2) * 64
                    for jb in range(NJB):
                        J0 = jb * 512
                        nkt_j = 4 * (jb + 1)
                        av = ps.tile([65, 512], f32, tag="av", name="av")
                        for kp in range(nkt_j // 2):
                            kts = (2 * kp, 2 * kp + 1)
                            # exact-causal: within the diagonal band start
                            # scores at column kt*128 instead of J0
                            offs = [max(0, kt * 128 - J0) for kt in kts]
                            ws = [512 - o for o in offs]
                            col0 = [0, 512] if offs[0] == 0 else [0, ws[0]]
                            sc = ps.tile([128, 1024], f32, tag="sc", name="sc")
                            for i, kt in enumerate(kts):
                                nc.tensor.matmul(
                                    out=sc[:, col0[i] : col0[i] + ws[i]],
                                    lhsT=krot[hb][
                                        hr : hr + 64, kt * 128 : (kt + 1) * 128
                                    ],
                                    rhs=qrot[hb][
                                        hr : hr + 64, J0 + offs[i] : J0 + 512
                                    ],
                                    start=True, stop=True,
                                )
                            e = esb.tile([128, 1024], bf, tag="e")
                            wtot = col0[1] + ws[1]
                            nc.scalar.activation(
                                out=e[:, 0:wtot], in_=sc[:, 0:wtot],
                                func=AF.Exp, scale=0.125,
                            )
                            for i, kt in enumerate(kts):
                                if offs[i] or kt * 128 == J0:  # diagonal tile
                                    nc.gpsimd.tensor_tensor(
                                        out=e[:, col0[i] : col0[i] + ws[i]],
                                        in0=e[:, col0[i] : col0[i] + ws[i]],
                                        in1=m01_s[:, 384 : 384 + ws[i]],
                                        op=ALU.mult,
                                    )
                                nc.tensor.matmul(
                                    out=av[:, offs[i] : 512],
                                    lhsT=vw[kt][:, h * 65 : (h + 1) * 65],
                                    rhs=e[:, col0[i] : col0[i] + ws[i]],
                                    start=(kt == 0), stop=(kt == nkt_j - 1),
                                )
                        avc = tmpb.tile([65, 512], f32, tag="avc")
                        nc.vector.tensor_copy(out=avc[:], in_=av[:])
                        rcp = tmpb.tile([1, 512], bf, tag="rcp")
                        act_recip(rcp[:], avc[64:65, :])
                        bc = ps.tile([64, 512], f32, tag="av", name="bc")
                        nc.tensor.matmul(
                            out=bc[:], lhsT=ones_bb[:], rhs=rcp[:],
                            start=True, stop=True,
                        )
                        nc.vector.tensor_tensor(
                            out=ot[hb][hr : hr + 64, J0 : J0 + 512],
                            in0=avc[0:64, :], in1=bc[:], op=ALU.mult,
                        )

                def oproj():
                    for jt in range(S // 128):
                        jsl = slice(jt * 128, (jt + 1) * 128)
                        po = ps.tile([128, D], f32, tag="sc", name="po")
                        for half in range(2):
                            osl = slice(half * 512, (half + 1) * 512)
                            if not zero_bias:
                                nc.tensor.matmul(
                                    out=po[:, osl],
                                    lhsT=ones_b[0:1, 0:128],
                                    rhs=bo_s[0:1, osl],
                                    start=True, stop=False,
                                )
                            for dt_i in range(2):
                                nc.tensor.matmul(
                                    out=po[:, osl],
                                    lhsT=ot[dt_i][:, jsl],
                                    rhs=wo_s[:, dt_i * D + half * 512 : dt_i * D + (half + 1) * 512],
                                    start=(zero_bias and dt_i == 0),
                                    stop=(dt_i == 1),
                                )
                        ob = ost.tile([128, D], f32, tag="ob")
                        nc.vector.tensor_copy(out=ob[:], in_=po[:])
                        nc.sync.dma_start(out=out_d[jsl, :], in_=ob[:])

                # order: blk0 projections + vw, then attention h0 first so
                # blk1 projection MMs act as PE filler during h0/h1 exp stalls
                proj_rope(qT_s, wq_s, bq_s, qrot, 0)
                proj_rope(kT_s, wk_s, bk_s, krot, 0)
                vw_proj()
                attention(0)
                proj_rope(qT_s, wq_s, bq_s, qrot, 1)
                proj_rope(kT_s, wk_s, bk_s, krot, 1)
                attention(1)
                attention(2)
                attention(3)
                oproj()

    return nc


def _get_nc(zero_bias=True):
    key = ("nc", zero_bias)
    if key not in _CACHE:
        _CACHE[key] = build_nc(zero_bias)
    return _CACHE[key]


def _host_prep(inputs):
    import ml_dtypes

    bf = ml_dtypes.bfloat16
    q, k, v = inputs["q"], inputs["k"], inputs["v"]
    rope, v_mask = inputs["rope"], inputs["v_mask"]
    Wq, bq = inputs["Wq"], inputs["bq"]
    Wk, bk = inputs["Wk"], inputs["bk"]
    Wv, bv = inputs["Wv"], inputs["bv"]
    Wo, bo = inputs["Wo"], inputs["bo"]

    def pack_w(w, width):  # [1024, G*width] -> [128, NDT*width]
        return np.ascontiguousarray(
            w.reshape(NDT, 128, width).transpose(1, 0, 2).reshape(128, NDT * width)
        ).astype(bf)

    # pairwise rotation matrix: qw2 = R2^T-applied; R2[2i+1, 2i]=-1, R2[2i,2i+1]=1
    R2 = np.zeros((128, 128), np.float32)
    idx = np.arange(0, 128, 2)
    R2[idx + 1, idx] = -1.0
    R2[idx, idx + 1] = 1.0
    R2 = R2.astype(bf)

    t = np.arange(896, dtype=np.float32)[None, :]
    p = np.arange(128, dtype=np.float32)[:, None]
    M01 = (t >= p + 384.0).astype(np.float32).astype(bf)

    in_maps = []
    for core in range(NCORES):
        b, g = core // GROUPS, core % GROUPS
        gsl = slice(g * DG, (g + 1) * DG)
        cos64 = np.repeat(rope[b, :, 1::2].T, 2, axis=0)  # [64, S]
        sin64 = np.repeat(rope[b, :, 0::2].T, 2, axis=0)
        wv_g = Wv[:, gsl]
        wv_pack = np.zeros((D, 260), np.float32)
        bva = np.zeros((1, 260), np.float32)
        for h in range(HPG):
            wv_pack[:, h * 65 : h * 65 + 64] = wv_g[:, h * 64 : (h + 1) * 64]
            bva[0, h * 65 : h * 65 + 64] = bv[gsl][h * 64 : (h + 1) * 64]
            bva[0, h * 65 + 64] = 1.0
        wo_g = Wo[gsl, :]
        in_maps.append({
            "qT": np.ascontiguousarray(q[b].T).astype(bf),
            "kT": np.ascontiguousarray(k[b].T).astype(bf),
            "vT": np.ascontiguousarray(v[b].T).astype(bf),
            "wq": pack_w(Wq[:, gsl], DG),
            "wk": pack_w(Wk[:, gsl], DG),
            "wv": pack_w(wv_pack, 260),
            "wo": np.ascontiguousarray(
                wo_g.reshape(2, 128, D).transpose(1, 0, 2).reshape(128, 2 * D)
            ).astype(bf),
            "r2": R2,
            "cosT": np.vstack([cos64, cos64]).astype(bf),
            "sinT": np.vstack([sin64, sin64]).astype(bf),
            "m01": M01,
            "bq_l": bq[gsl].reshape(1, DG).astype(bf),
            "bk_l": bk[gsl].reshape(1, DG).astype(bf),
            "bv_aug": bva.astype(bf),
            "bo_r": (bo / GROUPS).reshape(1, D).astype(bf),
            "vm": np.ascontiguousarray(v_mask[b].reshape(NKT, 128).T).astype(
                np.float32
            ),
        })
    return in_maps


def kernel(**inputs):
    from concourse.bass_utils import run_bass_kernel_spmd

    zero_bias = all(
        not np.any(inputs[k]) for k in ("bq", "bk", "bo")
    )
    nc = _get_nc(zero_bias)
    in_maps = _host_prep(inputs)
    res = run_bass_kernel_spmd(nc, in_maps, list(range(NCORES)))
    outs = [res.results[i]["out"] for i in range(NCORES)]
    full = np.empty((B, S, D), np.float32)
    for b in range(B):
        full[b] = sum(
            outs[b * GROUPS + g].astype(np.float32) for g in range(GROUPS)
        )
    return full
